# revision 78
# baseline (speedup 1.0000x reference)
"""DeepSpeedMLP Trainium2 kernel.

Computation (per reference):
    x   = input + bias + residual
    h   = LayerNorm(x) * ln_w + attn_nb          (ln_w == ones)
    h1  = relu(h @ inter_w + inter_b)
    out = h1 @ output_w + output_b + x

Sharding: pure data parallel over tokens. B*S = 8192 tokens split across
8 cores (1024 tokens each); weights replicated (cast to bf16 host-side).
attn_nb is folded into b1 on the host (relu((h+a)@W1+b1) ==
relu(h@W1 + (b1 + a@W1))), so the kernel never sees it.

Per-core dataflow (_build_v2; matmuls bf16 with fp32 PSUM accumulation).
The PE runs nothing but the fc1/fc2 GEMM streams -- all transposes go
through the DMA XBAR (dma_start(transpose=True)) and every other op is
placed on DVE / ACT / GPSIMD so the tensor engine never waits on them.
Tokens are processed in two 512-token halves (phase1 of half h+1 runs
under the matmuls of half h):
  phase 1: per [128-token, 1024-col] strip: io loads (SP queue, split
           SP/ACT for the very first half), x = in + res + bias on DVE
           (bf16), x -> bf16 DRAM scratch via the GPSIMD SWDGE queue,
           bn_stats/bn_aggr + rsqrt, normalized rows DMA-XBAR-transposed
           into xT [H, tok-half] bf16 from the ACT queue.
  fc1:     h1T[I-chunk, tok] = relu(W1-chunk.T @ xT + b1'), weight
           stationary over 16 H-chunks; ACT drains PSUM with relu+bias+
           bf16 cast. The first half of the first body runs the leading
           16 m-iterations in two 256-token quarters so the PE starts as
           soon as token blocks 0-1 are normalized.
  fc2:     outT[H-chunk, tok] = W2-chunk.T @ h1T over 64 I-chunks (w2
           streamed in 2 chunks per m2 on SP); ACT drains PSUM with
           +output_b and bf16 cast, DMA-XBAR-transposes into oT[tok, H];
           after every 4 (last groups: 2) output chunks, DVE adds the
           re-read x and results stream out on alternating SP/GPSIMD
           queues, keeping the drain off the critical tail.
"""

import numpy as np
import ml_dtypes

_B, _S, _H, _I = 4, 2048, 2048, 8192
_NCORES = 8
_LN_EPS = 1e-5

_CACHE = {}


def _build(TOK, H, I, repeat=1, flip=False):
    """Build + compile the per-core Bass kernel. Returns the compiled Bacc.

    flip=True: fc2 runs token-stationary (lhsT = h1T block, rhs = W2 rows)
    producing [tok, H] directly -- no output transposes, 2x fewer PE
    weight loads in fc2. flip=False keeps the weight-stationary fc2 with
    PE transposes of the output.
    """
    from contextlib import ExitStack

    import concourse.bass as bass
    import concourse.mybir as mybir
    import concourse.tile as tile
    from concourse import bacc
    from concourse.masks import make_identity

    f32 = mybir.dt.float32
    bf16 = mybir.dt.bfloat16
    Alu = mybir.AluOpType
    Act = mybir.ActivationFunctionType

    P = 128
    Hk = H // P          # H chunks (fc1 contraction / xT partition tiles)
    Im = I // P          # I chunks (fc1 output tiles / fc2 contraction)
    TB = TOK // P        # token blocks
    M2 = H // P          # fc2 output chunks
    NH = TOK // 2        # tokens per half == matmul free dim, <= 512
    TBH = TB // 2        # token blocks per half
    assert NH <= 512 and TB % 2 == 0
    SG = max(H // 512, 1)  # bn_stats subgroups

    nc = bacc.Bacc("TRN2", target_bir_lowering=False, debug=False)

    x_in = nc.dram_tensor("x_in", [TOK, H], f32, kind="ExternalInput")
    r_in = nc.dram_tensor("r_in", [TOK, H], f32, kind="ExternalInput")
    w1 = nc.dram_tensor("w1", [Im, P, Hk, P], bf16, kind="ExternalInput")
    b1 = nc.dram_tensor("b1", [P, Im], f32, kind="ExternalInput")
    if flip:
        w2 = nc.dram_tensor("w2", [Im, P, H], bf16, kind="ExternalInput")
        b2_row = nc.dram_tensor("b2_row", [1, H], bf16, kind="ExternalInput")
    else:
        w2 = nc.dram_tensor("w2", [M2, P, Im, P], bf16, kind="ExternalInput")
        b2_t = nc.dram_tensor("b2_t", [P, M2], f32, kind="ExternalInput")
    bias_v = nc.dram_tensor("bias_v", [H], f32, kind="ExternalInput")
    attn_t = nc.dram_tensor("attn_t", [P, Hk], f32, kind="ExternalInput")
    out_d = nc.dram_tensor("out", [TOK, H], f32, kind="ExternalOutput")
    xpb_d = nc.dram_tensor("xpb_scratch", [TOK, H], f32)

    def brd(vec_ap):  # broadcast a [H] dram vector across 128 partitions
        return bass.AP(
            tensor=vec_ap.tensor, offset=vec_ap.offset, ap=[[0, P], *vec_ap.ap]
        )

    with tile.TileContext(nc) as tc:
        with ExitStack() as st:
            consts = st.enter_context(tc.tile_pool(name="consts", bufs=1))
            ident_f32 = consts.tile([P, P], f32)
            make_identity(nc, ident_f32)
            ident_bf = consts.tile([P, P], bf16)
            make_identity(nc, ident_bf)
            b1_sb = consts.tile([P, Im], f32)
            nc.sync.dma_start(out=b1_sb, in_=b1[:])
            attn_sb = consts.tile([P, Hk], f32)
            nc.sync.dma_start(out=attn_sb, in_=attn_t[:])
            if flip:
                b2_sb = consts.tile([1, H], bf16)
                nc.sync.dma_start(out=b2_sb, in_=b2_row[:])
                ones_sb = consts.tile([1, P], bf16)
                nc.vector.memset(ones_sb, 1.0)
            else:
                b2_sb = consts.tile([P, M2], f32)
                nc.sync.dma_start(out=b2_sb, in_=b2_t[:])
            eps_sb = consts.tile([P, 1], f32)
            nc.vector.memset(eps_sb, _LN_EPS)
            bias_rep = consts.tile([P, H], f32)
            nc.sync.dma_start(out=bias_rep, in_=brd(bias_v[:]))

            big = st.enter_context(tc.tile_pool(name="big", bufs=1))
            io = st.enter_context(tc.tile_pool(name="io", bufs=3))
            lnp = st.enter_context(tc.tile_pool(name="lnp", bufs=2))
            w1p = st.enter_context(tc.tile_pool(name="w1p", bufs=4))
            w2p = st.enter_context(tc.tile_pool(name="w2p", bufs=3))
            xpbp = st.enter_context(tc.tile_pool(name="xpbp", bufs=2))
            outp = st.enter_context(tc.tile_pool(name="outp", bufs=4))
            PS = bass.MemorySpace.PSUM
            ps_tr = st.enter_context(tc.tile_pool(name="ps_tr", bufs=2, space=PS))
            ps_m1 = st.enter_context(tc.tile_pool(name="ps_m1", bufs=2, space=PS))
            if flip:
                ps_m2 = st.enter_context(
                    tc.tile_pool(name="ps_f2", bufs=TBH, space=PS)
                )
            else:
                ps_m2 = st.enter_context(tc.tile_pool(name="ps_m2", bufs=2, space=PS))

            xpb_r = xpb_d[:].rearrange("(j p) h -> p j h", p=P)

            for half in range(2 * repeat):
                half = half % 2
                t0 = half * NH

                # ---- phase 1: x, LayerNorm, transpose into xT ----
                xT = big.tile([P, Hk, NH], bf16, tag="xT")
                for jh in range(TBH):
                    j = half * TBH + jh
                    it = io.tile([P, H], f32, tag="io")
                    nc.sync.dma_start(out=it, in_=x_in[j * P:(j + 1) * P, :])
                    rt = io.tile([P, H], f32, tag="io")
                    nc.sync.dma_start(out=rt, in_=r_in[j * P:(j + 1) * P, :])

                    xt = lnp.tile([P, H], f32, tag="x")
                    nc.vector.tensor_add(out=xt, in0=it, in1=rt)
                    nc.vector.tensor_add(out=xt, in0=xt, in1=bias_rep)
                    nc.sync.dma_start(out=xpb_d[j * P:(j + 1) * P, :], in_=xt)

                    stats = lnp.tile([P, SG, 6], f32, tag="stats")
                    xg = xt.rearrange("p (n f) -> p n f", n=SG)
                    for g in range(SG):
                        nc.vector.bn_stats(out=stats[:, g, :], in_=xg[:, g, :])
                    mv = lnp.tile([P, 2], f32, tag="mv")
                    nc.vector.bn_aggr(out=mv, in_=stats)
                    rstd = lnp.tile([P, 1], f32, tag="rstd")
                    nc.scalar.activation(
                        out=rstd, in_=mv[:, 1:2], func=Act.Sqrt, bias=eps_sb
                    )
                    nc.vector.reciprocal(out=rstd, in_=rstd)

                    hf = lnp.tile([P, H], bf16, tag="hf")
                    nc.vector.tensor_scalar(
                        out=hf,
                        in0=xt,
                        scalar1=mv[:, 0:1],
                        scalar2=rstd,
                        op0=Alu.subtract,
                        op1=Alu.mult,
                    )
                    for k in range(Hk):
                        pt = ps_tr.tile([P, P], bf16, tag="pt")
                        nc.tensor.transpose(
                            out=pt,
                            in_=hf[:, k * P:(k + 1) * P],
                            identity=ident_bf,
                        )
                        # xT = pt + attn_nb (per-partition here); cast to bf16
                        nc.scalar.activation(
                            out=xT[:, k, jh * P:(jh + 1) * P],
                            in_=pt,
                            func=Act.Identity,
                            bias=attn_sb[:, k:k + 1],
                        )

                # ---- fc1 on this half ----
                h1T = big.tile([P, Im, NH], bf16, tag="h1T")
                for m in range(Im):
                    w1t = w1p.tile([P, Hk, P], bf16, tag="w1")
                    nc.sync.dma_start(out=w1t, in_=w1[m])
                    ps = ps_m1.tile([P, NH], f32, tag="mm1")
                    for k in range(Hk):
                        nc.tensor.matmul(
                            ps,
                            lhsT=w1t[:, k, :],
                            rhs=xT[:, k, :],
                            start=(k == 0),
                            stop=(k == Hk - 1),
                        )
                    nc.scalar.activation(
                        out=h1T[:, m, :],
                        in_=ps,
                        func=Act.Relu,
                        bias=b1_sb[:, m:m + 1],
                        scale=1.0,
                    )

                # ---- fc2 on this half ----
                if flip:
                    # token-stationary: out[tok, H-quarter] accumulated over I;
                    # output_b added via a ones-row K=1 matmul into the group.
                    NQ = 512
                    for hq in range(H // NQ):
                        ho = hq * NQ
                        ps2 = [
                            ps_m2.tile([P, NQ], f32, tag="f2", name=f"ps2_{tb}")
                            for tb in range(TBH)
                        ]
                        for k2 in range(Im):
                            w2t = w2p.tile([P, NQ], bf16, tag="w2")
                            nc.sync.dma_start(
                                out=w2t, in_=w2[k2, :, ho:ho + NQ]
                            )
                            for tb in range(TBH):
                                nc.tensor.matmul(
                                    ps2[tb],
                                    lhsT=h1T[:, k2, tb * P:(tb + 1) * P],
                                    rhs=w2t,
                                    start=(k2 == 0),
                                    stop=False,
                                )
                        for tb in range(TBH):
                            nc.tensor.matmul(
                                ps2[tb],
                                lhsT=ones_sb,
                                rhs=b2_sb[:, ho:ho + NQ],
                                start=False,
                                stop=True,
                            )
                        for tb in range(TBH):
                            j = half * TBH + tb
                            xq = xpbp.tile([P, NQ], f32, tag="xq")
                            nc.sync.dma_start(
                                out=xq,
                                in_=xpb_d[j * P:(j + 1) * P, ho:ho + NQ],
                            )
                            ot = outp.tile([P, NQ], f32, tag="ot")
                            nc.vector.tensor_add(out=ot, in0=ps2[tb], in1=xq)
                            nc.sync.dma_start(
                                out=out_d[j * P:(j + 1) * P, ho:ho + NQ],
                                in_=ot,
                            )
                else:
                    for m2 in range(M2):
                        w2t = w2p.tile([P, Im, P], bf16, tag="w2")
                        nc.sync.dma_start(out=w2t, in_=w2[m2])
                        xpb_t = xpbp.tile([P, TBH, P], f32, tag="xpb3")
                        nc.sync.dma_start(
                            out=xpb_t,
                            in_=xpb_r[
                                :, half * TBH:(half + 1) * TBH, m2 * P:(m2 + 1) * P
                            ],
                        )
                        ps2 = ps_m2.tile([P, NH], f32, tag="mm2")
                        for k2 in range(Im):
                            nc.tensor.matmul(
                                ps2,
                                lhsT=w2t[:, k2, :],
                                rhs=h1T[:, k2, :],
                                start=(k2 == 0),
                                stop=(k2 == Im - 1),
                            )
                        for jh in range(TBH):
                            j = half * TBH + jh
                            # out^T chunk + output_b (per-partition here)
                            stg = outp.tile([P, P], f32, tag="stg")
                            nc.scalar.activation(
                                out=stg,
                                in_=ps2[:, jh * P:(jh + 1) * P],
                                func=Act.Identity,
                                bias=b2_sb[:, m2:m2 + 1],
                            )
                            pt2 = ps_tr.tile([P, P], f32, tag="pt")
                            nc.tensor.transpose(
                                out=pt2, in_=stg, identity=ident_f32
                            )
                            ot = outp.tile([P, P], f32, tag="ot")
                            nc.vector.tensor_add(
                                out=ot, in0=pt2, in1=xpb_t[:, jh, :]
                            )
                            nc.sync.dma_start(
                                out=out_d[j * P:(j + 1) * P, m2 * P:(m2 + 1) * P],
                                in_=ot,
                            )

    nc.compile()
    return nc


def _build_v2(TOK, H, I, repeat=1):
    """v2: PE runs GEMMs only. Phase-1 normalized rows reach xT via DMA
    XBAR transpose (no PE transpose); fc2 output drains to bf16 and DMA-
    transposes back to [tok, H]. attn_nb is folded into b1 host-side
    (relu((h+attn)@W1+b1) == relu(h@W1 + (b1 + attn@W1))). x is kept in a
    bf16 DRAM scratch. DMA issue is split across the SP and ACT queues.
    """
    from contextlib import ExitStack, nullcontext

    import concourse.bass as bass
    import concourse.mybir as mybir
    import concourse.tile as tile
    from concourse import bacc

    f32 = mybir.dt.float32
    bf16 = mybir.dt.bfloat16
    Alu = mybir.AluOpType
    Act = mybir.ActivationFunctionType

    P = 128
    Hk = H // P          # xT partition tiles / fc1 contraction chunks
    Im = I // P          # fc1 output tiles / fc2 contraction chunks
    TB = TOK // P        # token blocks
    M2 = H // P          # fc2 output chunks
    NH = TOK // 2        # tokens per half == matmul free dim, <= 512
    TBH = TB // 2        # token blocks per half
    assert NH <= 512 and TB % 2 == 0
    SG = max(H // 512, 1)  # bn_stats subgroups

    nc = bacc.Bacc("TRN2", target_bir_lowering=False, debug=False)

    x_in = nc.dram_tensor("x_in", [TOK, H], f32, kind="ExternalInput")
    r_in = nc.dram_tensor("r_in", [TOK, H], f32, kind="ExternalInput")
    w1 = nc.dram_tensor("w1", [Im, P, Hk, P], bf16, kind="ExternalInput")
    b1 = nc.dram_tensor("b1", [P, Im], f32, kind="ExternalInput")
    w2 = nc.dram_tensor("w2", [M2, P, Im, P], bf16, kind="ExternalInput")
    b2_t = nc.dram_tensor("b2_t", [P, M2], f32, kind="ExternalInput")
    bias_v = nc.dram_tensor("bias_v", [H], f32, kind="ExternalInput")
    out_d = nc.dram_tensor("out", [TOK, H], f32, kind="ExternalOutput")
    xpb_d = nc.dram_tensor("xpb_scratch", [TOK, H], bf16)

    def brd(vec_ap):  # broadcast a [H] dram vector across 128 partitions
        return bass.AP(
            tensor=vec_ap.tensor, offset=vec_ap.offset, ap=[[0, P], *vec_ap.ap]
        )

    with tile.TileContext(nc) as tc:
        with ExitStack() as st:
            consts = st.enter_context(tc.tile_pool(name="consts", bufs=1))
            bias_rep = consts.tile([P, H], f32)
            for qi in range(4):
                sl = slice(qi * H // 4, (qi + 1) * H // 4)
                nc.gpsimd.dma_start(out=bias_rep[:, sl], in_=brd(bias_v[sl]))
            b1_sb = consts.tile([P, Im], f32)
            nc.gpsimd.dma_start(out=b1_sb, in_=b1[:])
            b2_sb = consts.tile([P, M2], f32)
            nc.gpsimd.dma_start(out=b2_sb, in_=b2_t[:])
            eps_sb = consts.tile([P, 1], f32)
            nc.vector.memset(eps_sb, _LN_EPS)

            big = st.enter_context(tc.tile_pool(name="big", bufs=1))
            io = st.enter_context(tc.tile_pool(name="io", bufs=6))
            lnp = st.enter_context(tc.tile_pool(name="lnp", bufs=2))
            w1p = st.enter_context(tc.tile_pool(name="w1p", bufs=4))
            w2p = st.enter_context(tc.tile_pool(name="w2p", bufs=3))
            stgp = st.enter_context(tc.tile_pool(name="stgp", bufs=3))
            xpbp = st.enter_context(tc.tile_pool(name="xpbp", bufs=5))
            outp = st.enter_context(tc.tile_pool(name="outp", bufs=4))
            PS = bass.MemorySpace.PSUM
            ps_m1 = st.enter_context(tc.tile_pool(name="ps_m1", bufs=3, space=PS))
            ps_m2 = st.enter_context(tc.tile_pool(name="ps_m2", bufs=3, space=PS))

            for it_idx in range(2 * repeat):
                half = it_idx % 2
                first = it_idx == 0
                t0 = half * NH

                # ---- phase 1: x = in + res + bias; LN; DMA-transpose ----
                # Processed in [128, 1024] half-row strips so io / DVE /
                # transpose work pipelines in small quanta across both DMA
                # queues instead of monolithic 8KB-per-partition steps.
                H2 = H // 2
                Hk2 = Hk // 2
                xT = big.tile([P, Hk, NH], bf16, tag="xT")
                for jh in range(TBH):
                    j = half * TBH + jh
                    xts = []
                    SGS = max(SG // 2, 1)  # bn_stats subgroups per strip
                    stats = lnp.tile([P, 2 * SGS, 6], f32, tag="stats")
                    for s in range(2):
                        sl = slice(s * H2, (s + 1) * H2)
                        rt_eng = nc.scalar if first else nc.sync
                        it = io.tile([P, H2], f32, tag="io")
                        nc.sync.dma_start(out=it, in_=x_in[j * P:(j + 1) * P, sl])
                        rt = io.tile([P, H2], f32, tag="io")
                        rt_eng.dma_start(out=rt, in_=r_in[j * P:(j + 1) * P, sl])

                        nc.vector.tensor_add(out=it, in0=it, in1=rt)
                        xt = lnp.tile([P, H2], bf16, tag="x")
                        nc.vector.tensor_add(out=xt, in0=it, in1=bias_rep[:, sl])
                        nc.gpsimd.dma_start(
                            out=xpb_d[j * P:(j + 1) * P, sl], in_=xt
                        )
                        xts.append(xt)

                        xg = xt.rearrange("p (n f) -> p n f", n=SGS)
                        for g in range(SGS):
                            nc.vector.bn_stats(
                                out=stats[:, s * SGS + g, :], in_=xg[:, g, :]
                            )
                    mv = lnp.tile([P, 2], f32, tag="mv")
                    nc.vector.bn_aggr(out=mv, in_=stats)
                    rstd = lnp.tile([P, 1], f32, tag="rstd")
                    nc.scalar.activation(
                        out=rstd, in_=mv[:, 1:2], func=Act.Sqrt, bias=eps_sb
                    )
                    nc.vector.reciprocal(out=rstd, in_=rstd)

                    for s in range(2):
                        hf = lnp.tile([P, H2], bf16, tag="hf")
                        nc.vector.tensor_scalar(
                            out=hf,
                            in0=xts[s],
                            scalar1=mv[:, 0:1],
                            scalar2=rstd,
                            op0=Alu.subtract,
                            op1=Alu.mult,
                        )
                        # hf [tok, H-half] -> xT chunk via DMA XBAR transpose.
                        # NOTE: all XBAR transposes must stay on ONE queue --
                        # alternating them across SP+ACT corrupts results on
                        # real HW (shared XBAR state; sim does not model it).
                        nc.scalar.dma_start(
                            out=xT[:, s * Hk2:(s + 1) * Hk2,
                                   jh * P:(jh + 1) * P],
                            in_=hf,
                            transpose=True,
                        )

                # ---- fc1 on this half ----
                # In the first body, the first 8 m-iterations run in two
                # 256-token quarters (reloading those w1 chunks once) so the
                # PE starts once token blocks 0-1 are normalized instead of
                # waiting for the whole half's transposes.
                h1T = big.tile([P, Im, NH], bf16, tag="h1T")

                def fc1_piece(m, q0, qn):
                    w1t = w1p.tile([P, Hk, P], bf16, tag="w1")
                    nc.sync.dma_start(out=w1t, in_=w1[m])
                    ps = ps_m1.tile([P, qn - q0], f32, tag="mm1")
                    for k in range(Hk):
                        nc.tensor.matmul(
                            ps,
                            lhsT=w1t[:, k, :],
                            rhs=xT[:, k, q0:qn],
                            start=(k == 0),
                            stop=(k == Hk - 1),
                        )
                    nc.scalar.activation(
                        out=h1T[:, m, q0:qn],
                        in_=ps,
                        func=Act.Relu,
                        bias=b1_sb[:, m:m + 1],
                        scale=1.0,
                    )

                Q1M = min(20, Im)
                if first:
                    for m in range(Q1M):
                        fc1_piece(m, 0, NH // 2)
                    for m in range(Q1M):
                        fc1_piece(m, NH // 2, NH)
                    for m in range(Q1M, Im):
                        fc1_piece(m, 0, NH)
                else:
                    for m in range(Im):
                        fc1_piece(m, 0, NH)

                # ---- fc2 on this half (drain+store per 4-chunk group) ----
                # x re-reads for the final residual add; they complete on
                # the Pool queue well before the drain groups need them.
                xqs = []
                for jh in range(TBH):
                    j = half * TBH + jh
                    xq = xpbp.tile([P, H], bf16, tag="xq")
                    nc.gpsimd.dma_start(out=xq, in_=xpb_d[j * P:(j + 1) * P, :])
                    xqs.append(xq)

                oT = big.tile([P, TBH, H], bf16, tag="oT")
                drain_after = {3: 4, 7: 4, 11: 4, 13: 2, 14: 1, 15: 1}
                Im2 = Im // 2
                for m2 in range(M2):
                    # In the first body, keep the early w2 prefetches off the
                    # DMA rings until phase 1 is done with them (the loads
                    # are not needed until fc2 at ~240us).
                    gate = tc.tile_wait_until(0.05, enable=(first and m2 < 4))
                    w2a = w2p.tile([P, Im2, P], bf16, tag="w2")
                    with gate:
                        nc.sync.dma_start(out=w2a, in_=w2[m2, :, :Im2, :])
                        w2b = w2p.tile([P, Im2, P], bf16, tag="w2")
                        nc.sync.dma_start(out=w2b, in_=w2[m2, :, Im2:, :])
                    ps2 = ps_m2.tile([P, NH], f32, tag="mm2")
                    for k2 in range(Im):
                        wt = w2a if k2 < Im2 else w2b
                        nc.tensor.matmul(
                            ps2,
                            lhsT=wt[:, k2 % Im2, :],
                            rhs=h1T[:, k2, :],
                            start=(k2 == 0),
                            stop=(k2 == Im - 1),
                        )
                    # + output_b (per-partition here); cast to bf16
                    stg = stgp.tile([P, NH], bf16, tag="stg")
                    nc.scalar.activation(
                        out=stg,
                        in_=ps2,
                        func=Act.Identity,
                        bias=b2_sb[:, m2:m2 + 1],
                    )
                    # [H-chunk, tok] -> oT [tok, H-chunk] via DMA transpose
                    nc.scalar.dma_start(
                        out=oT[:, :, m2 * P:(m2 + 1) * P], in_=stg, transpose=True
                    )
                    gw = drain_after.get(m2)
                    if gw:
                        # residual add + store for the finished column group
                        c0, c1 = (m2 - gw + 1) * P, (m2 + 1) * P
                        for jh in range(TBH):
                            j = half * TBH + jh
                            ot = outp.tile([P, gw * P], f32, tag="ot")
                            nc.vector.tensor_add(
                                out=ot,
                                in0=oT[:, jh, c0:c1],
                                in1=xqs[jh][:, c0:c1],
                            )
                            eng = nc.sync if jh % 2 == 0 else nc.gpsimd
                            eng.dma_start(
                                out=out_d[j * P:(j + 1) * P, c0:c1], in_=ot
                            )

    nc.compile()
    return nc


def _prep_weights_v2(inter_w, inter_b, output_w, attn_nb, output_b):
    P = 128
    H, I = inter_w.shape
    Hk, Im, M2 = H // P, I // P, H // P
    bf = ml_dtypes.bfloat16
    w1 = np.ascontiguousarray(
        inter_w.reshape(Hk, P, Im, P).transpose(2, 1, 0, 3)
    ).astype(bf)
    b1f = inter_b.astype(np.float64) + attn_nb.astype(np.float64) @ inter_w.astype(
        np.float64
    )
    b1 = np.ascontiguousarray(b1f.reshape(Im, P).T).astype(np.float32)
    w2 = np.ascontiguousarray(
        output_w.reshape(Im, P, M2, P).transpose(2, 1, 0, 3)
    ).astype(bf)
    b2 = np.ascontiguousarray(output_b.reshape(M2, P).T).astype(np.float32)
    return w1, b1, w2, b2


def _get_compiled(TOK=None, H=None, I=None):
    key = (TOK or _B * _S // _NCORES, H or _H, I or _I)
    if key not in _CACHE:
        _CACHE[key] = _build_v2(*key)
    return _CACHE[key]


def _prep_weights(inter_w, inter_b, output_w, attn_nb, output_b, flip=False):
    P = 128
    H, I = inter_w.shape
    Hk, Im, M2 = H // P, I // P, H // P
    bf = ml_dtypes.bfloat16
    w1 = np.ascontiguousarray(
        inter_w.reshape(Hk, P, Im, P).transpose(2, 1, 0, 3)
    ).astype(bf)
    b1 = np.ascontiguousarray(inter_b.reshape(Im, P).T).astype(np.float32)
    attn_t = np.ascontiguousarray(attn_nb.reshape(Hk, P).T).astype(np.float32)
    if flip:
        w2 = np.ascontiguousarray(output_w.reshape(Im, P, H)).astype(bf)
        b2 = np.ascontiguousarray(output_b.reshape(1, H)).astype(bf)
    else:
        w2 = np.ascontiguousarray(
            output_w.reshape(Im, P, M2, P).transpose(2, 1, 0, 3)
        ).astype(bf)
        b2 = np.ascontiguousarray(output_b.reshape(M2, P).T).astype(np.float32)
    return w1, b1, w2, attn_t, b2


def kernel(**inputs):
    inp = np.asarray(inputs["input"], np.float32)
    res = np.asarray(inputs["residual"], np.float32)
    bias = np.asarray(inputs["bias"], np.float32)
    attn_nb = np.asarray(inputs["attn_nb"], np.float32)
    inter_w = np.asarray(inputs["inter_w"], np.float32)
    inter_b = np.asarray(inputs["inter_b"], np.float32)
    output_w = np.asarray(inputs["output_w"], np.float32)
    output_b = np.asarray(inputs["output_b"], np.float32)
    # residual_norm, weight, ln_w are unused by the reference computation
    # (ln_w is all-ones).

    B, S, H = inp.shape
    N = B * S
    TOK = N // _NCORES

    from concourse.bass_utils import run_bass_kernel_spmd

    nc = _get_compiled(TOK, H, inter_w.shape[1])
    w1, b1, w2, b2 = _prep_weights_v2(
        inter_w, inter_b, output_w, attn_nb, output_b
    )

    xf = np.ascontiguousarray(inp.reshape(N, H))
    rf = np.ascontiguousarray(res.reshape(N, H))
    in_maps = []
    for c in range(_NCORES):
        in_maps.append(
            {
                "x_in": xf[c * TOK:(c + 1) * TOK],
                "r_in": rf[c * TOK:(c + 1) * TOK],
                "w1": w1,
                "b1": b1,
                "w2": w2,
                "bias_v": bias,
                "b2_t": b2,
            }
        )
    results = run_bass_kernel_spmd(nc, in_maps, core_ids=list(range(_NCORES)))
    out = np.concatenate([results.results[c]["out"] for c in range(_NCORES)], axis=0)
    return out.reshape(B, S, H).astype(np.float32)

